# Initial kernel scaffold
#
"""Trainium2 Bass kernel for nn_Block_23338852286694 (dense transformer block).

Sharding: 8 cores = 4 batches x 2 query-halves. Inputs are rotated per core so
each core's 512 query tokens are tokens 0:512 of its (rotated) sequence; K/V
work over the full 1024-token sequence is duplicated across the pair of cores
sharing a batch (no collectives). Softmax over keys is permutation-invariant,
so rotation does not change results.

v2 structural changes vs baseline:
- ~35 big DMAs instead of ~550 small ones (each dma_start costs ~0.6-1us on
  the shared HWDGE path); weights stream as [128,nk,768]-unit DMAs issued from
  the Pool engine (SWDGE) to stay off the HWDGE.
- No DRAM round-trips: partition-broadcasts of LN stats and softmax
  reciprocals go through K=1 matmuls into PSUM; attention outputs land in
  SBUF via partition-offset-shifted DVE writes (av psum[0:64] -> sbuf
  [64:128]); intermediates never stage through DRAM.
- bf16 for all matmul operands except the residual spine (x residual, aT,
  asum, final combine stay f32). PSUM always f32.
"""
import numpy as np

import concourse.bass as bass
import concourse.bacc as bacc
import concourse.mybir as mybir
import concourse.tile as tile
from concourse.bass_utils import run_bass_kernel_spmd

F32 = mybir.dt.float32
F32R = mybir.dt.float32r
BF16 = mybir.dt.bfloat16
AF = mybir.ActivationFunctionType
ALU = mybir.AluOpType

B, S, SE = 4, 1024, 1024
E, H, M, D = 768, 12, 100, 64
KC = E // 128            # 6 feature chunks
Q = S // 2               # 512 query tokens per core
EPS = 1e-5
NKT = S // 128           # 8 key tiles

# packed per-partition bias column offsets (see _pack_bias_cols)
BC_Q, BC_K, BC_PROJ, BC_MA, BC_FCQ, BC_FCK, BC_EP, BC_A1, BC_A2, BC_FC, BC_PJ = (
    0, 6, 12, 18, 24, 30, 36, 42, 48, 54, 78)
NB = 84


def _row_bcast_dram(ap, parts):
    """DRAM row [N] -> AP readable as [parts, N] (partition-broadcast)."""
    return bass.AP(tensor=ap.tensor, offset=ap.offset,
                   ap=[[0, parts], list(ap.ap[-1])])


def build_program():
    nc = bacc.Bacc(trn_type="TRN2")

    xT = nc.dram_tensor("xT", [E, S], BF16, kind="ExternalInput")
    xqf = nc.dram_tensor("xqf", [E, Q], F32, kind="ExternalInput")
    encT = nc.dram_tensor("encT", [2, E, SE], BF16, kind="ExternalInput")
    maskmul = nc.dram_tensor("maskmul", [128, NKT], F32, kind="ExternalInput")
    mkT = nc.dram_tensor("mkT", [128, KC, M], BF16, kind="ExternalInput")
    mvA = nc.dram_tensor("mvA", [M, H * 65], BF16, kind="ExternalInput")
    w_qk = nc.dram_tensor("w_qk", [E, 2 * E], BF16, kind="ExternalInput")
    w_vs = nc.dram_tensor("w_vs", [E, E], BF16, kind="ExternalInput")
    w_proj = nc.dram_tensor("w_proj", [E, E], BF16, kind="ExternalInput")
    w_ma = nc.dram_tensor("w_ma", [2 * E, E], BF16, kind="ExternalInput")
    w_q = nc.dram_tensor("w_q", [E, E], BF16, kind="ExternalInput")
    w_k = nc.dram_tensor("w_k", [E, E], BF16, kind="ExternalInput")
    w_v = nc.dram_tensor("w_v", [E, E], BF16, kind="ExternalInput")
    w_ep = nc.dram_tensor("w_ep", [E, E], BF16, kind="ExternalInput")
    w_a1 = nc.dram_tensor("w_a1", [2 * E, E], BF16, kind="ExternalInput")
    w_a2 = nc.dram_tensor("w_a2", [2 * E, E], BF16, kind="ExternalInput")
    w_fc = nc.dram_tensor("w_fc", [E, 4 * E], BF16, kind="ExternalInput")
    w_pj = nc.dram_tensor("w_pj", [4 * E, E], BF16, kind="ExternalInput")
    bcols = nc.dram_tensor("bcols", [128, NB], F32, kind="ExternalInput")
    bv_self = nc.dram_tensor("bv_self", [E], F32, kind="ExternalInput")
    bv_enc = nc.dram_tensor("bv_enc", [E], F32, kind="ExternalInput")
    outT = nc.dram_tensor("outT", [E, Q], F32, kind="ExternalOutput")

    wdr = {"qk": w_qk, "vs": w_vs, "proj": w_proj, "ma": w_ma, "q": w_q,
           "k": w_k, "v": w_v, "ep": w_ep, "a1": w_a1, "a2": w_a2,
           "fc": w_fc, "pj": w_pj}

    with tile.TileContext(nc) as tc:
        _emit(nc, tc, xT, xqf, encT, maskmul, mkT, mvA, wdr, bcols,
              bv_self, bv_enc, outT)
    nc.compile()
    return nc


def _emit(nc, tc, xT, xqf, encT, maskmul, mkT, mvA, wdr, bcols,
          bv_self, bv_enc, outT):
    from contextlib import ExitStack
    ctx = ExitStack()
    with ctx:
        consts = ctx.enter_context(tc.tile_pool(name="consts", bufs=1))
        bigp = ctx.enter_context(tc.tile_pool(name="big", bufs=2))
        vsbp = ctx.enter_context(tc.tile_pool(name="vsb", bufs=1))
        kchp = ctx.enter_context(tc.tile_pool(name="kch", bufs=2))
        ptp = ctx.enter_context(tc.tile_pool(name="pt", bufs=3))
        qp = ctx.enter_context(tc.tile_pool(name="qp", bufs=2))
        g24 = ctx.enter_context(tc.tile_pool(name="g24", bufs=1))
        aTp = ctx.enter_context(tc.tile_pool(name="aTp", bufs=1))
        asp = ctx.enter_context(tc.tile_pool(name="asp", bufs=1))
        n12 = ctx.enter_context(tc.tile_pool(name="n12", bufs=2))
        msp = ctx.enter_context(tc.tile_pool(name="msp", bufs=1))
        wup = ctx.enter_context(tc.tile_pool(name="wup", bufs=4))
        rcbp = ctx.enter_context(tc.tile_pool(name="rcbp", bufs=2))
        t32 = ctx.enter_context(tc.tile_pool(name="t32", bufs=4))
        tbf = ctx.enter_context(tc.tile_pool(name="tbf", bufs=2))
        rowp = ctx.enter_context(tc.tile_pool(name="rowp", bufs=3))
        rcpp = ctx.enter_context(tc.tile_pool(name="rcpp", bufs=2))
        brdp = ctx.enter_context(tc.tile_pool(name="brdp", bufs=2))
        pln = ctx.enter_context(tc.tile_pool(name="pln", bufs=2, space="PSUM"))
        pst = ctx.enter_context(tc.tile_pool(name="pst", bufs=2, space="PSUM"))
        psc = ctx.enter_context(tc.tile_pool(name="psc", bufs=2, space="PSUM"))
        pav = ctx.enter_context(tc.tile_pool(name="pav", bufs=2, space="PSUM"))

        # ---- constants ----
        onesf = consts.tile([128, 12], F32)
        nc.vector.memset(onesf, 1.0)
        ones_r = consts.tile([128, 1], F32R)      # f32r stats lhsT
        nc.vector.tensor_copy(out=ones_r, in_=onesf[:, 0:1])
        ones_b = consts.tile([128, 1], BF16)      # bf16 stats lhsT
        nc.vector.tensor_copy(out=ones_b, in_=onesf[:, 0:1])
        ones128 = consts.tile([1, 128], F32)
        nc.vector.memset(ones128, 1.0)
        onerow = consts.tile([1, 128], F32R)      # K=1 broadcast lhsT
        nc.vector.tensor_copy(out=onerow, in_=ones128)
        bc = consts.tile([128, NB], F32)
        nc.sync.dma_start(out=bc, in_=bcols[:, :])
        mm_sb = consts.tile([128, NKT], F32)
        nc.sync.dma_start(out=mm_sb, in_=maskmul[:, :])
        mk_sb = consts.tile([128, KC, M], BF16)
        nc.sync.dma_start(out=mk_sb, in_=mkT[:, :, :])
        mv_sb = consts.tile([M, H * 65], BF16)
        nc.sync.dma_start(out=mv_sb, in_=mvA[:, :])
        bvb_s = consts.tile([128, E], F32)
        nc.sync.dma_start(out=bvb_s, in_=_row_bcast_dram(bv_self[:], 128))
        bvb_e = consts.tile([128, E], F32)
        nc.sync.dma_start(out=bvb_e, in_=_row_bcast_dram(bv_enc[:], 128))
        eps_t = consts.tile([128, 1], F32)
        nc.vector.memset(eps_t, EPS)

        def wunit(wkey, kc0, nk, f0, nf, name):
            t = wup.tile([128, nk, nf], BF16, tag="wu", name=name)
            src = wdr[wkey][:, :].rearrange("(c p) f -> p c f", p=128)
            nc.gpsimd.dma_start(out=t, in_=src[:, kc0:kc0 + nk, f0:f0 + nf])
            return t

        def stats_apply(src_fn, T, out_fn, sdt):
            """Feature-major layernorm: out = (src - mu) * rsqrt(var+eps).

            src_fn(c) -> AP [128, T] (dtype sdt: BF16 or F32);
            out_fn(c) -> AP [128, T] (any dtype, bf16 typical).
            Processed in 512-token halves (tokens are independent).
            """
            ones = ones_b if sdt == BF16 else ones_r
            bdt = BF16 if sdt == BF16 else F32
            for th in range(T // 512):
                sl = slice(th * 512, (th + 1) * 512)
                ps_s = pst.tile([1, 512], F32, tag="pst", name="ps_s")
                ps_q = pst.tile([1, 512], F32, tag="pst", name="ps_q")
                for c in range(KC):
                    src = src_fn(c)[:, sl]
                    src_f = src if sdt == BF16 else src.bitcast(F32)
                    sq = t32.tile([128, 512], F32R, tag="t32", name="sq")
                    nc.scalar.activation(out=sq, in_=src_f,
                                         func=AF.Square, scale=1.0)
                    nc.tensor.matmul(ps_s[:, :], ones, src,
                                     start=(c == 0), stop=(c == KC - 1))
                    nc.tensor.matmul(ps_q[:, :], ones_r, sq,
                                     start=(c == 0), stop=(c == KC - 1))
                mu = rowp.tile([1, 512], F32, tag="row", name="mu")
                nc.vector.tensor_scalar(out=mu, in0=ps_s[:, :],
                                        scalar1=1.0 / E, scalar2=None,
                                        op0=ALU.mult)
                var = rowp.tile([1, 512], F32, tag="row", name="var")
                nc.vector.tensor_tensor(out=var, in0=mu, in1=mu, op=ALU.mult)
                nc.vector.scalar_tensor_tensor(
                    out=var, in0=ps_q[:, :], scalar=1.0 / E, in1=var,
                    op0=ALU.mult, op1=ALU.subtract)
                nc.scalar.activation(out=var, in_=var, func=AF.Sqrt,
                                     bias=eps_t[0:1, :], scale=1.0)
                rs = rowp.tile([1, 512], F32R, tag="row", name="rs")
                with nc.allow_low_precision(reason="f32r feed to bcast mm"):
                    nc.vector.reciprocal(out=rs, in_=var)
                murs = rowp.tile([1, 512], F32R, tag="row", name="murs")
                nc.vector.tensor_tensor(out=murs, in0=mu,
                                        in1=rs.bitcast(F32), op=ALU.mult)
                ps_rs = pst.tile([128, 512], F32, tag="pst", name="ps_rs")
                nc.tensor.matmul(ps_rs[:, :], onerow, rs, start=True, stop=True)
                ps_mu = pst.tile([128, 512], F32, tag="pst", name="ps_mu")
                nc.tensor.matmul(ps_mu[:, :], onerow, murs, start=True,
                                 stop=True)
                rs_b = brdp.tile([128, 512], bdt, tag="brd", name="rs_b")
                nc.scalar.activation(out=rs_b, in_=ps_rs[:, :],
                                     func=AF.Identity, scale=1.0)
                mu_b = brdp.tile([128, 512], bdt, tag="brd", name="mu_b")
                nc.scalar.activation(out=mu_b, in_=ps_mu[:, :],
                                     func=AF.Identity, scale=1.0)
                for c in range(KC):
                    src = src_fn(c)[:, sl]
                    src_f = src if sdt == BF16 else src.bitcast(F32)
                    t1 = t32.tile([128, 512], bdt, tag="t32", name="t1")
                    nc.vector.tensor_tensor(out=t1, in0=src_f, in1=rs_b,
                                            op=ALU.mult)
                    nc.gpsimd.tensor_tensor(out=out_fn(c)[:, sl], in0=t1,
                                            in1=mu_b, op=ALU.subtract)

        def linear(out_fn, rhs_fn, wt, nk, fts, T, bias_col, act, kcw0=0,
                   ftw0=0, dve_bias=False):
            """out_fn(i)[128,T] = act(sum_kc wt[:,kc,:]^T rhs_fn(kc) + bias).

            fts: list of ft indices (output 128-chunks); wt cols indexed
            relative to ftw0."""
            nt = T // 512
            for i, ft in enumerate(fts):
                for t in range(nt):
                    sl = slice(t * 512, (t + 1) * 512)
                    ps = pln.tile([128, 512], F32, tag="lin", name="linps")
                    for kc in range(nk):
                        wslice = wt[:, kcw0 + kc,
                                    (ft - ftw0) * 128:(ft - ftw0 + 1) * 128]
                        nc.tensor.matmul(ps[:, :], wslice, rhs_fn(kc)[:, sl],
                                         start=(kc == 0), stop=(kc == nk - 1))
                    if dve_bias:
                        nc.vector.tensor_scalar(
                            out=out_fn(i)[:, sl], in0=ps[:, :],
                            scalar1=bc[:, bias_col + ft:bias_col + ft + 1],
                            scalar2=None, op0=ALU.add)
                    else:
                        nc.scalar.activation(
                            out=out_fn(i)[:, sl], in_=ps[:, :], func=act,
                            bias=bc[:, bias_col + ft:bias_col + ft + 1],
                            scale=1.0)

        def make_v(v_tile, src_fn, wv, bias_b, masked):
            for tt in range(NKT):
                for h0 in (0, 6):
                    ps = pln.tile([128, 512], F32, tag="lin", name="vps")
                    for kc in range(KC):
                        nc.tensor.matmul(ps[:, 0:384],
                                         src_fn(kc)[:, tt * 128:(tt + 1) * 128],
                                         wv[:, kc, h0 * 64:h0 * 64 + 384],
                                         start=(kc == 0), stop=(kc == KC - 1))
                    vrow = v_tile[:, tt, :].rearrange("p (h c) -> p h c", c=65)
                    nc.vector.tensor_tensor(
                        out=vrow[:, h0:h0 + 6, 0:64],
                        in0=ps[:, 0:384].rearrange("p (h c) -> p h c", c=64),
                        in1=bias_b[:, h0 * 64:h0 * 64 + 384].rearrange(
                            "p (h c) -> p h c", c=64),
                        op=ALU.add)
                    if masked:
                        nc.vector.tensor_scalar(
                            out=v_tile[:, tt, h0 * 65:(h0 + 6) * 65],
                            in0=v_tile[:, tt, h0 * 65:(h0 + 6) * 65],
                            scalar1=mm_sb[:, tt:tt + 1], scalar2=None,
                            op0=ALU.mult)

        def init_ones_cols(v_tile):
            for tt in range(NKT):
                vrow = v_tile[:, tt, :].rearrange("p (h c) -> p h c", c=65)
                nc.vector.tensor_copy(
                    out=vrow[:, :, 64:65],
                    in_=onesf[:, :].rearrange("p (h o) -> p h o", o=1))

        def norm_write(av, dst_ap):
            """Normalize AV psum [65,512] by its ones-row denominator and
            write dst_ap [64, 512] (possibly partition-shifted)."""
            rcp = rcpp.tile([1, 512], F32R, tag="rcp", name="rcp")
            with nc.allow_low_precision(reason="f32r feed to bcast mm"):
                nc.vector.reciprocal(out=rcp, in_=av[64:65, :])
            ps_rc = pst.tile([64, 512], F32, tag="pst", name="ps_rc")
            nc.tensor.matmul(ps_rc[:, :], onerow[:, 0:64], rcp,
                             start=True, stop=True)
            rcb = rcbp.tile([64, 512], F32, tag="rcb", name="rcb")
            nc.vector.tensor_scalar(out=rcb, in0=ps_rc[:, :], scalar1=1.0,
                                    scalar2=None, op0=ALU.mult)
            nc.vector.tensor_tensor(out=dst_ap, in0=av[0:64, :], in1=rcb,
                                    op=ALU.mult)

        def attention(kch_fn, v_tile, q_tile, dst, slot0, scale,
                      mem_dst_slot0=None):
            """q_tile [128, KC, Q] bf16; writes dst[off:off+64, slot0+c, :]."""
            for c in range(KC):
                kch = kch_fn(c)
                for hh in range(2):
                    h, off = 2 * c + hh, 64 * hh
                    av = pav.tile([65, 512], F32, tag="av", name="av")
                    for kt in range(NKT):
                        sc = psc.tile([128, 512], F32, tag="sc", name="sc")
                        nc.tensor.matmul(sc[:, :],
                                         kch[off:off + 64,
                                             kt * 128:(kt + 1) * 128],
                                         q_tile[off:off + 64, c, :],
                                         start=True, stop=True)
                        pt = ptp.tile([128, 512], BF16, tag="pt", name="pt")
                        nc.scalar.activation(out=pt, in_=sc[:, :], func=AF.Exp,
                                             scale=scale)
                        nc.tensor.matmul(av[:, :],
                                         v_tile[:, kt, h * 65:(h + 1) * 65],
                                         pt[:, :],
                                         start=(kt == 0), stop=(kt == NKT - 1))
                    norm_write(av, dst[off:off + 64, slot0 + c, :])
                    if mem_dst_slot0 is not None:
                        scm = psc.tile([128, 512], F32, tag="sc", name="scm")
                        nc.tensor.matmul(scm[0:M, :], mk_sb[off:off + 64, c, :],
                                         q_tile[off:off + 64, c, :],
                                         start=True, stop=True)
                        pmt = ptp.tile([128, 512], BF16, tag="pt", name="pmt")
                        nc.scalar.activation(out=pmt[0:M, :], in_=scm[0:M, :],
                                             func=AF.Exp, scale=1.0)
                        av1 = pav.tile([65, 512], F32, tag="av", name="av1")
                        nc.tensor.matmul(av1[:, :],
                                         mv_sb[:, h * 65:(h + 1) * 65],
                                         pmt[0:M, :], start=True, stop=True)
                        norm_write(av1,
                                   dst[off:off + 64, mem_dst_slot0 + c, :])

        # ======== phase A: x load + layernorm ========
        x_sb = bigp.tile([128, KC, 1024], BF16, tag="big", name="x_sb")
        xr = xT[:, :].rearrange("(c p) t -> p c t", p=128)
        for th in range(2):
            nc.sync.dma_start(out=x_sb[:, :, th * 512:(th + 1) * 512],
                              in_=xr[:, :, th * 512:(th + 1) * 512])
        xhat = bigp.tile([128, KC, 1024], BF16, tag="big", name="xhat")
        stats_apply(lambda c: x_sb[:, c, :], S, lambda c: xhat[:, c, :], BF16)
        xq32 = asp.tile([128, KC, Q], F32, tag="as", name="xq32")
        nc.sync.dma_start(out=xq32,
                          in_=xqf[:, :].rearrange("(c p) t -> p c t", p=128))

        # ======== phase B: V, q, self+memory attention ========
        v_sb = vsbp.tile([128, NKT, H * 65], BF16, tag="vsb")
        init_ones_cols(v_sb)
        wu_vs = wunit("vs", 0, KC, 0, E, "wu_vs")
        make_v(v_sb, lambda kc: xhat[:, kc, :], wu_vs, bvb_s, masked=False)
        wu_qq = wunit("qk", 0, KC, 0, E, "wu_qq")
        qT = qp.tile([128, KC, Q], BF16, tag="q", name="qT")
        linear(lambda i: qT[:, i, :], lambda kc: xhat[:, kc, 0:Q], wu_qq,
               KC, list(range(KC)), Q, BC_Q, AF.Identity)
        gAB = g24.tile([128, 12, Q], BF16, tag="g24", name="gAB")
        wu_qk = wunit("qk", 0, KC, E, E, "wu_qk")

        def self_kchunk(c):
            kt = kchp.tile([128, S], BF16, tag="kch", name="kchs")
            linear(lambda i: kt, lambda kc: xhat[:, kc, :], wu_qk, KC, [c], S,
                   BC_K, AF.Identity, dve_bias=True)
            return kt

        attention(self_kchunk, v_sb, qT, gAB, 0, 1.0, mem_dst_slot0=KC)

        # ======== phase B4: memory gate + attn_proj + residual ========
        wu_ma0 = wunit("ma", 0, KC, 0, E, "wu_ma0")
        wu_ma1 = wunit("ma", KC, KC, 0, E, "wu_ma1")
        aN = n12.tile([128, KC, Q], BF16, tag="n12", name="aN")
        for ft in range(KC):
            ps = pln.tile([128, 512], F32, tag="lin", name="maps")
            for kc in range(12):
                wt = wu_ma0 if kc < KC else wu_ma1
                nc.tensor.matmul(ps[:, :],
                                 wt[:, kc % KC, ft * 128:(ft + 1) * 128],
                                 gAB[:, kc, :], start=(kc == 0),
                                 stop=(kc == 11))
            al = tbf.tile([128, 512], BF16, tag="tbf", name="al")
            nc.scalar.activation(out=al, in_=ps[:, :], func=AF.Sigmoid,
                                 bias=bc[:, BC_MA + ft:BC_MA + ft + 1],
                                 scale=1.0)
            d = tbf.tile([128, 512], BF16, tag="tbf", name="d")
            nc.vector.tensor_tensor(out=d, in0=gAB[:, ft, :],
                                    in1=gAB[:, KC + ft, :], op=ALU.subtract)
            nc.vector.tensor_tensor(out=d, in0=al, in1=d, op=ALU.mult)
            nc.gpsimd.tensor_tensor(out=aN[:, ft, :], in0=gAB[:, KC + ft, :],
                                    in1=d, op=ALU.add)
        wu_pr = wunit("proj", 0, KC, 0, E, "wu_pr")
        aT = aTp.tile([128, KC, Q], F32R, tag="aT", name="aT")
        ee = g24.tile([128, 18, Q], BF16, tag="g24", name="ee")
        for ft in range(KC):
            ps = pln.tile([128, 512], F32, tag="lin", name="prps")
            for kc in range(KC):
                nc.tensor.matmul(ps[:, :],
                                 wu_pr[:, kc, ft * 128:(ft + 1) * 128],
                                 aN[:, kc, :], start=(kc == 0),
                                 stop=(kc == KC - 1))
            nc.vector.scalar_tensor_tensor(
                out=aT[:, ft, :], in0=ps[:, :],
                scalar=bc[:, BC_PROJ + ft:BC_PROJ + ft + 1],
                in1=xq32[:, ft, :], op0=ALU.add, op1=ALU.add)
            nc.scalar.activation(out=ee[:, ft, :],
                                 in_=aT[:, ft, :].bitcast(F32),
                                 func=AF.Identity, scale=1.0)

        # ======== phase C: LN(aT) -> qe ========
        hah = n12.tile([128, KC, Q], BF16, tag="n12", name="hah")
        stats_apply(lambda c: aT[:, c, :], Q, lambda c: hah[:, c, :], F32R)
        wu_q = wunit("q", 0, KC, 0, E, "wu_q")
        qeT = qp.tile([128, KC, Q], BF16, tag="q", name="qeT")
        linear(lambda i: qeT[:, i, :], lambda kc: hah[:, kc, :], wu_q,
               KC, list(range(KC)), Q, BC_FCQ, AF.Identity)

        asum = asp.tile([128, KC, Q], F32R, tag="as", name="asum")

        # ======== phase D: two cross-attentions ========
        for e in range(2):
            enc = bigp.tile([128, KC, 1024], BF16, tag="big", name="enc")
            nc.sync.dma_start(
                out=enc,
                in_=encT[e].rearrange("(c p) t -> p c t", p=128))
            stats_apply(lambda c: enc[:, c, :], SE, lambda c: enc[:, c, :],
                        BF16)
            wu_v = wunit("v", 0, KC, 0, E, f"wu_v{e}")
            make_v(v_sb, lambda kc: enc[:, kc, :], wu_v, bvb_e, masked=True)
            wu_k = wunit("k", 0, KC, 0, E, f"wu_k{e}")

            def enc_kchunk(c, _enc=enc, _wu_k=wu_k):
                kt = kchp.tile([128, SE], BF16, tag="kch", name="kche")
                linear(lambda i: kt, lambda kc: _enc[:, kc, :], _wu_k, KC,
                       [c], SE, BC_FCK, AF.Identity, dve_bias=True)
                return kt

            attention(enc_kchunk, v_sb, qeT, ee, KC, 0.125)

            wu_ep = wunit("ep", 0, KC, 0, E, f"wu_ep{e}")
            ep32 = n12.tile([128, KC, Q], F32, tag="n12", name="ep32")
            for ft in range(KC):
                ps = pln.tile([128, 512], F32, tag="lin", name="epps")
                for kc in range(KC):
                    nc.tensor.matmul(ps[:, :],
                                     wu_ep[:, kc, ft * 128:(ft + 1) * 128],
                                     ee[:, KC + kc, :], start=(kc == 0),
                                     stop=(kc == KC - 1))
                nc.vector.tensor_scalar(
                    out=ep32[:, ft, :], in0=ps[:, :],
                    scalar1=bc[:, BC_EP + ft:BC_EP + ft + 1], scalar2=None,
                    op0=ALU.add)
                nc.scalar.activation(out=ee[:, 2 * KC + ft, :],
                                     in_=ep32[:, ft, :], func=AF.Identity,
                                     scale=1.0)

            akey = "a1" if e == 0 else "a2"
            wu_g0 = wunit(akey, 0, KC, 0, E, f"wu_g0{e}")
            wu_g1 = wunit(akey, KC, KC, 0, E, f"wu_g1{e}")
            bcol0 = BC_A1 if e == 0 else BC_A2
            for ft in range(KC):
                ps = pln.tile([128, 512], F32, tag="lin", name="gps")
                for kc in range(12):
                    wt = wu_g0 if kc < KC else wu_g1
                    src = ee[:, kc, :] if kc < KC else ee[:, KC + kc, :]
                    nc.tensor.matmul(ps[:, :],
                                     wt[:, kc % KC, ft * 128:(ft + 1) * 128],
                                     src, start=(kc == 0), stop=(kc == 11))
                al = t32.tile([128, 512], F32, tag="t32", name="alE")
                nc.scalar.activation(out=al, in_=ps[:, :], func=AF.Sigmoid,
                                     bias=bc[:, bcol0 + ft:bcol0 + ft + 1],
                                     scale=1.0)
                # eg = ep + al*(aT - ep), all f32
                t = t32.tile([128, 512], F32, tag="t32", name="tE")
                nc.vector.tensor_tensor(out=t, in0=aT[:, ft, :].bitcast(F32),
                                        in1=ep32[:, ft, :], op=ALU.subtract)
                nc.vector.tensor_tensor(out=t, in0=al, in1=t, op=ALU.mult)
                if e == 0:
                    nc.gpsimd.tensor_tensor(out=asum[:, ft, :],
                                            in0=ep32[:, ft, :], in1=t,
                                            op=ALU.add)
                else:
                    eg = t32.tile([128, 512], F32, tag="t32", name="egE")
                    nc.vector.tensor_tensor(out=eg, in0=ep32[:, ft, :],
                                            in1=t, op=ALU.add)
                    nc.gpsimd.tensor_tensor(out=asum[:, ft, :],
                                            in0=asum[:, ft, :].bitcast(F32),
                                            in1=eg, op=ALU.add)

        # ======== phase E: MLP + final residual ========
        hm2 = n12.tile([128, KC, Q], BF16, tag="n12", name="hm2")
        stats_apply(lambda c: asum[:, c, :], Q, lambda c: hm2[:, c, :], F32R)
        mstage = msp.tile([128, KC, Q], F32, tag="ms", name="mstage")
        for mh in range(2):
            mT = g24.tile([128, 12, Q], BF16, tag="g24", name="mT")
            for u in range(2):
                wu_fc = wunit("fc", 0, KC, (2 * mh + u) * E, E,
                              f"wu_fc{mh}{u}")
                linear(lambda i, _u=u: mT[:, 6 * _u + i, :],
                       lambda kc: hm2[:, kc, :], wu_fc, KC, list(range(KC)),
                       Q, BC_FC + 12 * mh + 6 * u, AF.Gelu_apprx_tanh,
                       ftw0=0)
            wu_p0 = wunit("pj", 12 * mh, KC, 0, E, f"wu_p0{mh}")
            wu_p1 = wunit("pj", 12 * mh + KC, KC, 0, E, f"wu_p1{mh}")
            for ft in range(KC):
                ps = pln.tile([128, 512], F32, tag="lin", name="pjps")
                for kc in range(12):
                    wt = wu_p0 if kc < KC else wu_p1
                    nc.tensor.matmul(ps[:, :],
                                     wt[:, kc % KC, ft * 128:(ft + 1) * 128],
                                     mT[:, kc, :], start=(kc == 0),
                                     stop=(kc == 11))
                if mh == 0:
                    nc.scalar.activation(out=mstage[:, ft, :], in_=ps[:, :],
                                         func=AF.Identity,
                                         bias=bc[:, BC_PJ + ft:BC_PJ + ft + 1],
                                         scale=1.0)
                else:
                    t = t32.tile([128, 512], F32, tag="t32", name="mo")
                    nc.vector.scalar_tensor_tensor(
                        out=t, in0=asum[:, ft, :].bitcast(F32),
                        scalar=float(1.0 / np.sqrt(2.0)), in1=ps[:, :],
                        op0=ALU.mult, op1=ALU.add)
                    ot = t32.tile([128, 512], F32, tag="t32", name="ot")
                    nc.vector.tensor_tensor(out=ot, in0=t, in1=mstage[:, ft, :],
                                            op=ALU.add)
                    nc.sync.dma_start(out=outT[ft * 128:(ft + 1) * 128, :],
                                      in_=ot)


_NC_CACHE = None


def _get_nc():
    global _NC_CACHE
    if _NC_CACHE is None:
        _NC_CACHE = build_program()
    return _NC_CACHE


def _pack_bias_cols(seg_biases):
    bcols = np.zeros((128, NB), np.float32)
    for col0, b in seg_biases:
        nf = b.shape[0] // 128
        bcols[:, col0:col0 + nf] = b.reshape(nf, 128).T
    return bcols


def _bf16(x):
    import ml_dtypes
    return np.asarray(x, np.float32).astype(ml_dtypes.bfloat16).view(np.uint16)


def kernel(x, encoder_features, mask_encoder, ln1_g, ln1_b, ln2_g, ln2_b,
           c_attn_w, c_attn_b, attn_proj_w, attn_proj_b,
           memory_features, mem_attn_w, mem_attn_b, mem_alpha_w, mem_alpha_b,
           fcq_w, fcq_b, fck_w, fck_b, fcv_w, fcv_b, enc_proj_w, enc_proj_b,
           fc_alpha1_w, fc_alpha1_b, fc_alpha2_w, fc_alpha2_b,
           mlp_fc_w, mlp_fc_b, mlp_proj_w, mlp_proj_b):
    f32 = np.float32
    x = np.asarray(x, f32)
    encoder_features = np.asarray(encoder_features, f32)

    # ---- fold LN gains/biases into consumer weights ----
    g1 = np.asarray(ln1_g, f32); b1 = np.asarray(ln1_b, f32)
    g2 = np.asarray(ln2_g, f32); b2 = np.asarray(ln2_b, f32)

    def fold(w, b, g, lb):
        w = np.asarray(w, f32); b = np.asarray(b, f32)
        return (w * g[:, None]).astype(f32), (lb @ w + b).astype(f32)

    w_qkv, b_qkv = fold(c_attn_w, c_attn_b, g1, b1)
    w_fcq, b_fcq = fold(fcq_w, fcq_b, g1, b1)
    w_fck, b_fck = fold(fck_w, fck_b, g1, b1)
    w_fcv, b_fcv = fold(fcv_w, fcv_b, g1, b1)
    w_mfc, b_mfc = fold(mlp_fc_w, mlp_fc_b, g2, b2)

    # ---- memory slots (batch independent) ----
    mem = (np.asarray(memory_features, f32)[0] @ np.asarray(mem_attn_w, f32)
           + np.asarray(mem_attn_b, f32))          # [M, 2E]
    mk = mem[:, :E].reshape(M, H, D)
    mv = mem[:, E:].reshape(M, H, D)
    mkT = np.zeros((128, KC, M), f32)
    mvA = np.zeros((M, H * 65), f32)
    for h in range(H):
        c, off = divmod(h, 2)
        mkT[off * 64:(off + 1) * 64, c, :] = mk[:, h, :].T
        mvA[:, h * 65:h * 65 + 64] = mv[:, h, :]
        mvA[:, h * 65 + 64] = 1.0

    bcols = _pack_bias_cols([
        (BC_Q, b_qkv[0:E]), (BC_K, b_qkv[E:2 * E]),
        (BC_PROJ, np.asarray(attn_proj_b, f32)),
        (BC_MA, np.asarray(mem_alpha_b, f32)),
        (BC_FCQ, b_fcq), (BC_FCK, b_fck),
        (BC_EP, np.asarray(enc_proj_b, f32)),
        (BC_A1, np.asarray(fc_alpha1_b, f32)),
        (BC_A2, np.asarray(fc_alpha2_b, f32)),
        (BC_FC, b_mfc), (BC_PJ, np.asarray(mlp_proj_b, f32)),
    ])

    keep = (~np.asarray(mask_encoder, bool)[:, 0, 0, :]).astype(f32)  # [B, SE]

    common = dict(
        mkT=_bf16(mkT), mvA=_bf16(mvA),
        w_qk=_bf16(w_qkv[:, 0:2 * E]),
        w_vs=_bf16(w_qkv[:, 2 * E:3 * E]),
        w_proj=_bf16(attn_proj_w),
        w_ma=_bf16(mem_alpha_w),
        w_q=_bf16(w_fcq), w_k=_bf16(w_fck), w_v=_bf16(w_fcv),
        w_ep=_bf16(enc_proj_w),
        w_a1=_bf16(fc_alpha1_w),
        w_a2=_bf16(fc_alpha2_w),
        w_fc=_bf16(w_mfc), w_pj=_bf16(mlp_proj_w),
        bcols=bcols,
        bv_self=np.ascontiguousarray(b_qkv[2 * E:3 * E]),
        bv_enc=b_fcv,
    )

    in_maps = []
    for core in range(8):
        b, half = divmod(core, 2)
        xTb = np.ascontiguousarray(x[b].T)                       # [E, S]
        xrot = np.concatenate([xTb[:, half * Q:], xTb[:, :half * Q]], axis=1)
        m = dict(common)
        m["xT"] = _bf16(xrot)
        m["xqf"] = np.ascontiguousarray(xrot[:, 0:Q])
        m["encT"] = _bf16(encoder_features[b].transpose(0, 2, 1))
        m["maskmul"] = np.ascontiguousarray(keep[b].reshape(NKT, 128).T)
        in_maps.append(m)

    nc = _get_nc()
    res = run_bass_kernel_spmd(nc, in_maps, core_ids=list(range(8)))

    global _LAST_IN_MAPS
    _LAST_IN_MAPS = in_maps

    y = np.empty((B, S, E), f32)
    for core in range(8):
        b, half = divmod(core, 2)
        y[b, half * Q:(half + 1) * Q, :] = res.results[core]["outT"].T
    return y


_LAST_IN_MAPS = None


def profile_exec_ns(n_hot=12, n_cold=2):
    """Estimate per-invocation device time by timing pipelined repeats of the
    jitted 8-core executable with device-resident inputs."""
    import time
    import jax
    from jax.sharding import Mesh, PartitionSpec
    from jax.experimental.shard_map import shard_map
    import concourse.mybir as mybir_
    from concourse import bass2jax

    if _LAST_IN_MAPS is None:
        return None
    nc = _get_nc()
    in_maps = _LAST_IN_MAPS
    n_cores = 8
    bass2jax.install_neuronx_cc_hook()

    in_names, out_names, out_avals, zero_outs = [], [], [], []
    partition_name = nc.partition_id_tensor.name if nc.partition_id_tensor else None
    for alloc in nc.m.functions[0].allocations:
        if not isinstance(alloc, mybir_.MemoryLocationSet):
            continue
        name = alloc.memorylocations[0].name
        if alloc.kind == "ExternalInput":
            if name != partition_name:
                in_names.append(name)
        elif alloc.kind == "ExternalOutput":
            out_avals.append(jax.core.ShapedArray(
                tuple(alloc.tensor_shape), mybir_.dt.np(alloc.dtype)))
            zero_outs.append(np.zeros(tuple(alloc.tensor_shape),
                                      mybir_.dt.np(alloc.dtype)))
            out_names.append(name)
    n_params = len(in_names)
    n_outs = len(out_avals)
    all_in_names = in_names + out_names + ([partition_name] if partition_name else [])
    donate = tuple(range(n_params, n_params + n_outs))

    def _body(*args):
        operands = list(args)
        if partition_name is not None:
            operands.append(bass2jax.partition_id_tensor())
        return tuple(bass2jax._bass_exec_p.bind(
            *operands, out_avals=tuple(out_avals), in_names=tuple(all_in_names),
            out_names=tuple(out_names), lowering_input_output_aliases=(),
            sim_require_finite=True, sim_require_nnan=True, nc=nc))

    devices = jax.devices()[:n_cores]
    mesh = Mesh(np.asarray(devices), ("core",))
    fn = jax.jit(shard_map(_body, mesh=mesh,
                           in_specs=(PartitionSpec("core"),) * (n_params + n_outs),
                           out_specs=(PartitionSpec("core"),) * n_outs,
                           check_rep=False),
                 donate_argnums=donate, keep_unused=True)
    sh = jax.sharding.NamedSharding(mesh, PartitionSpec("core"))
    concat_in = [jax.device_put(
        np.concatenate([np.asarray(in_maps[c][nm]) for c in range(n_cores)], 0), sh)
        for nm in in_names]

    def zeros():
        return [jax.device_put(
            np.zeros((n_cores * z.shape[0], *z.shape[1:]), z.dtype), sh)
            for z in zero_outs]

    def run(n):
        o = tuple(zeros())
        o = fn(*concat_in, *o)
        jax.block_until_ready(o)
        t0 = time.perf_counter()
        for _ in range(n):
            o = fn(*concat_in, *o)
        jax.block_until_ready(o)
        return time.perf_counter() - t0

    tc = run(n_cold)
    th = run(n_hot)
    per = (th - tc) / (n_hot - n_cold)
    print(f"pipelined wall: {n_cold} calls {tc*1e3:.2f} ms, "
          f"{n_hot} calls {th*1e3:.2f} ms -> per-call {per*1e6:.0f} us")
    return int(per * 1e9)



# revision 120
# speedup vs baseline: 3.8631x; 3.8631x over previous
"""Trainium2 Bass kernel for nn_Block_23338852286694 (dense transformer block).

Sharding: 8 cores = 4 batches x 2 query-halves. Inputs are rotated per core so
each core's 512 query tokens are tokens 0:512 of its (rotated) sequence; K/V
work over the full 1024-token sequence is duplicated across the pair of cores
sharing a batch (no collectives). Softmax over keys is permutation-invariant,
so rotation does not change results.

v3 structural changes:
- ~35 big DMAs, all on the sync (HWDGE) queue in consumption order: x, enc
  prefetch, weight units, f32 biases/residual. Weight units are [128,nk,768]
  bf16 tiles in a 3-deep ring.
- No DRAM round-trips: partition-broadcasts of LN stats and softmax
  reciprocals go through K=1 matmuls into PSUM; intermediates never stage
  through DRAM.
- LN group (x + both encoders) is software-pipelined: per-job stats matmuls
  overlap the previous job's DVE row chain; x's apply on DVE immediately,
  enc0's on DVE after the last rows, enc1's on the Pool engine in the
  background. Squares alternate Act/DVE so stats aren't Act-paced. All PSUM
  rings sized to exactly 8 banks; pln is exclusively linears so LN broadcast
  readers never gate linear-chain slot reuse.
- Act-table preloads (tiny activations) hoist the 1.28us table loads for
  Exp/Sigmoid/Sqrt/Gelu off the LN/gate critical paths.
- bf16 for all matmul operands except the residual spine (x residual, aT,
  asum, final combine stay f32). PSUM always f32.
"""
import numpy as np

import concourse.bass as bass
import concourse.bacc as bacc
import concourse.mybir as mybir
import concourse.tile as tile
from concourse.bass_utils import run_bass_kernel_spmd

F32 = mybir.dt.float32
F32R = mybir.dt.float32r
BF16 = mybir.dt.bfloat16
AF = mybir.ActivationFunctionType
ALU = mybir.AluOpType

B, S, SE = 4, 1024, 1024
E, H, M, D = 768, 12, 100, 64
KC = E // 128            # 6 feature chunks
Q = S // 2               # 512 query tokens per core
EPS = 1e-5
NKT = S // 128           # 8 key tiles

# packed per-partition bias column offsets (see _pack_bias_cols)
BC_Q, BC_K, BC_PROJ, BC_MA, BC_FCQ, BC_FCK, BC_EP, BC_A1, BC_A2, BC_FC, BC_PJ = (
    0, 6, 12, 18, 24, 30, 36, 42, 48, 54, 78)
NB = 84


def _row_bcast_dram(ap, parts):
    """DRAM row [N] -> AP readable as [parts, N] (partition-broadcast)."""
    return bass.AP(tensor=ap.tensor, offset=ap.offset,
                   ap=[[0, parts], list(ap.ap[-1])])


def build_program(stop_after=None, repeat=1):
    nc = bacc.Bacc(trn_type="TRN2")

    xT = nc.dram_tensor("xT", [E, S], BF16, kind="ExternalInput")
    xqf = nc.dram_tensor("xqf", [E, Q], F32, kind="ExternalInput")
    encT = nc.dram_tensor("encT", [2, E, SE], BF16, kind="ExternalInput")
    maskmul = nc.dram_tensor("maskmul", [128, NKT], F32, kind="ExternalInput")
    mkT = nc.dram_tensor("mkT", [128, KC, M], BF16, kind="ExternalInput")
    mvA = nc.dram_tensor("mvA", [M, H * 65], BF16, kind="ExternalInput")
    w_qk = nc.dram_tensor("w_qk", [E, 2 * E], BF16, kind="ExternalInput")
    w_vs = nc.dram_tensor("w_vs", [E, E], BF16, kind="ExternalInput")
    w_proj = nc.dram_tensor("w_proj", [E, E], BF16, kind="ExternalInput")
    w_ma = nc.dram_tensor("w_ma", [2 * E, E], BF16, kind="ExternalInput")
    w_q = nc.dram_tensor("w_q", [E, E], BF16, kind="ExternalInput")
    w_k = nc.dram_tensor("w_k", [E, E], BF16, kind="ExternalInput")
    w_v = nc.dram_tensor("w_v", [E, E], BF16, kind="ExternalInput")
    w_ep = nc.dram_tensor("w_ep", [E, E], BF16, kind="ExternalInput")
    w_a1 = nc.dram_tensor("w_a1", [2 * E, E], BF16, kind="ExternalInput")
    w_a2 = nc.dram_tensor("w_a2", [2 * E, E], BF16, kind="ExternalInput")
    w_fc = nc.dram_tensor("w_fc", [E, 4 * E], BF16, kind="ExternalInput")
    w_pj = nc.dram_tensor("w_pj", [4 * E, E], BF16, kind="ExternalInput")
    bcols = nc.dram_tensor("bcols", [128, NB], F32, kind="ExternalInput")
    bv_self = nc.dram_tensor("bv_self", [E], F32, kind="ExternalInput")
    bv_enc = nc.dram_tensor("bv_enc", [E], F32, kind="ExternalInput")
    outT = nc.dram_tensor("outT", [E, Q], F32, kind="ExternalOutput")

    wdr = {"qk": w_qk, "vs": w_vs, "proj": w_proj, "ma": w_ma, "q": w_q,
           "k": w_k, "v": w_v, "ep": w_ep, "a1": w_a1, "a2": w_a2,
           "fc": w_fc, "pj": w_pj}

    with tile.TileContext(nc) as tc:
        for _ in range(repeat):
            _emit(nc, tc, xT, xqf, encT, maskmul, mkT, mvA, wdr, bcols,
                  bv_self, bv_enc, outT, stop_after)
    nc.compile()
    return nc


def _emit(nc, tc, xT, xqf, encT, maskmul, mkT, mvA, wdr, bcols,
          bv_self, bv_enc, outT, stop_after=None):
    from contextlib import ExitStack
    ctx = ExitStack()
    with ctx:
        consts = ctx.enter_context(tc.tile_pool(name="consts", bufs=1))
        bigp = ctx.enter_context(tc.tile_pool(name="big", bufs=1))
        encp = ctx.enter_context(tc.tile_pool(name="encp", bufs=2))
        vsbp = ctx.enter_context(tc.tile_pool(name="vsb", bufs=1))
        kchp = ctx.enter_context(tc.tile_pool(name="kch", bufs=2))
        ptp = ctx.enter_context(tc.tile_pool(name="pt", bufs=3))
        qp = ctx.enter_context(tc.tile_pool(name="qp", bufs=2))
        g24 = ctx.enter_context(tc.tile_pool(name="g24", bufs=1))
        aTp = ctx.enter_context(tc.tile_pool(name="aTp", bufs=1))
        asp = ctx.enter_context(tc.tile_pool(name="asp", bufs=1))
        n12 = ctx.enter_context(tc.tile_pool(name="n12", bufs=2))
        msp = ctx.enter_context(tc.tile_pool(name="msp", bufs=1))
        wup = ctx.enter_context(tc.tile_pool(name="wup", bufs=3))
        rcbp = ctx.enter_context(tc.tile_pool(name="rcbp", bufs=2))
        t32 = ctx.enter_context(tc.tile_pool(name="t32", bufs=3))
        tbf = ctx.enter_context(tc.tile_pool(name="tbf", bufs=2))
        rowp = ctx.enter_context(tc.tile_pool(name="rowp", bufs=3))
        rcpp = ctx.enter_context(tc.tile_pool(name="rcpp", bufs=2))
        brdp = ctx.enter_context(tc.tile_pool(name="brdp", bufs=4))
        pln = ctx.enter_context(tc.tile_pool(name="pln", bufs=2, space="PSUM"))
        pst = ctx.enter_context(tc.tile_pool(name="pst", bufs=2, space="PSUM"))
        psc = ctx.enter_context(tc.tile_pool(name="psc", bufs=2, space="PSUM"))
        pav = ctx.enter_context(tc.tile_pool(name="pav", bufs=2, space="PSUM"))

        # ---- input loads in consumption order: x, enc, weights, rest ----
        x_sb = bigp.tile([128, KC, 1024], BF16, tag="big", name="x_sb")
        xr = xT[:, :].rearrange("(c p) t -> p c t", p=128)
        for th in range(2):
            nc.sync.dma_start(out=x_sb[:, :, th * 512:(th + 1) * 512],
                              in_=xr[:, :, th * 512:(th + 1) * 512])
        enc_sb = []
        for e in range(2):
            t = encp.tile([128, KC, 1024], BF16, tag="enc", name=f"enc{e}")
            nc.sync.dma_start(out=t,
                              in_=encT[e].rearrange("(c p) t -> p c t", p=128))
            enc_sb.append(t)

        # ---- constants ----
        onesf = consts.tile([128, 12], F32)
        nc.vector.memset(onesf, 1.0)
        ones_r = consts.tile([128, 1], F32R)      # f32r stats lhsT
        nc.vector.tensor_copy(out=ones_r, in_=onesf[:, 0:1])
        ones_b = consts.tile([128, 1], BF16)      # bf16 stats lhsT
        nc.vector.tensor_copy(out=ones_b, in_=onesf[:, 0:1])
        ones128 = consts.tile([1, 128], F32)
        nc.vector.memset(ones128, 1.0)
        onerow = consts.tile([1, 128], F32R)      # K=1 broadcast lhsT
        nc.vector.tensor_copy(out=onerow, in_=ones128)
        bc = consts.tile([128, NB], F32)
        nc.sync.dma_start(out=bc, in_=bcols[:, :])
        mm_sb = consts.tile([128, NKT], F32)
        nc.sync.dma_start(out=mm_sb, in_=maskmul[:, :])
        mk_sb = consts.tile([128, KC, M], BF16)
        nc.sync.dma_start(out=mk_sb, in_=mkT[:, :, :])
        mv_sb = consts.tile([M, H * 65], BF16)
        nc.sync.dma_start(out=mv_sb, in_=mvA[:, :])
        eps_t = consts.tile([128, 1], F32)
        nc.vector.memset(eps_t, EPS)
        scratch1 = consts.tile([1, 1], F32)

        def preload(func):
            """Tiny activation to hoist the act-table load off the critical
            path: the LOAD the framework inserts before this op runs while
            PE is still busy with the preceding dense region."""
            nc.scalar.activation(out=scratch1, in_=eps_t[0:1, :], func=func,
                                 scale=1.0)

        def wunit(wkey, kc0, nk, f0, nf, name):
            t = wup.tile([128, nk, nf], BF16, tag="wu", name=name)
            src = wdr[wkey][:, :].rearrange("(c p) f -> p c f", p=128)
            nc.sync.dma_start(out=t, in_=src[:, kc0:kc0 + nk, f0:f0 + nf])
            return t

        def stats_apply(*jobs):
            """Feature-major layernorm: out = (src - mu) * rsqrt(var+eps).

            Each job is (src_fn, T, out_fn, sdt); src_fn(c) -> AP [128, T].
            Emission: per job stats matmuls + row chain + bcasts (job j+1's
            stats overlap job j's DVE row chain), then ALL applies last so
            the DVE apply burst never HOL-blocks a later job's bcast.
            """
            def stats(job):
                src_fn, T, out_fn, sdt = job
                ones = ones_b if sdt == BF16 else ones_r
                ps = []
                for th in range(T // 512):
                    sl = slice(th * 512, (th + 1) * 512)
                    ps_s = pst.tile([1, 512], F32, tag="pst", name="ps_s")
                    ps_q = pst.tile([1, 512], F32, tag="pst", name="ps_q")
                    for c in range(KC):
                        src = src_fn(c)[:, sl]
                        src_f = src if sdt == BF16 else src.bitcast(F32)
                        sq = t32.tile([128, 512], F32R, tag="t32", name="sq")
                        if c % 2 == 0:
                            nc.scalar.activation(out=sq, in_=src_f,
                                                 func=AF.Square, scale=1.0)
                        else:
                            nc.vector.tensor_tensor(out=sq, in0=src_f,
                                                    in1=src_f, op=ALU.mult)
                        nc.tensor.matmul(ps_s[:, :], ones, src,
                                         start=(c == 0), stop=(c == KC - 1))
                        nc.tensor.matmul(ps_q[:, :], ones_r, sq,
                                         start=(c == 0), stop=(c == KC - 1))
                    ps.append((ps_s, ps_q))
                return ps

            def rows_bcast(job, ps):
                src_fn, T, out_fn, sdt = job
                brd = []
                for th in range(T // 512):
                    ps_s, ps_q = ps[th]
                    mu = rowp.tile([1, 512], F32, tag="row", name="mu")
                    nc.vector.tensor_scalar(out=mu, in0=ps_s[:, :],
                                            scalar1=1.0 / E, scalar2=None,
                                            op0=ALU.mult)
                    var = rowp.tile([1, 512], F32, tag="row", name="var")
                    nc.vector.tensor_tensor(out=var, in0=mu, in1=mu,
                                            op=ALU.mult)
                    nc.vector.scalar_tensor_tensor(
                        out=var, in0=ps_q[:, :], scalar=1.0 / E, in1=var,
                        op0=ALU.mult, op1=ALU.subtract)
                    nc.scalar.activation(out=var, in_=var, func=AF.Sqrt,
                                         bias=eps_t[0:1, :], scale=1.0)
                    rs = rowp.tile([1, 512], F32R, tag="row", name="rs")
                    with nc.allow_low_precision(reason="f32r feed to bcast"):
                        nc.vector.reciprocal(out=rs, in_=var)
                    murs = rowp.tile([1, 512], F32R, tag="row", name="murs")
                    nc.vector.tensor_tensor(out=murs, in0=mu,
                                            in1=rs.bitcast(F32), op=ALU.mult)
                    ps_rs = pst.tile([128, 512], F32, tag="pst", name="ps_rs")
                    nc.tensor.matmul(ps_rs[:, :], onerow, rs, start=True,
                                     stop=True)
                    ps_mu = pst.tile([128, 512], F32, tag="pst", name="ps_mu")
                    nc.tensor.matmul(ps_mu[:, :], onerow, murs, start=True,
                                     stop=True)
                    # Act-engine copies (Pool cannot read PSUM). With applies
                    # placed a0-after-r0 / a1-after-r2 / a2-on-Pool-last, no
                    # Act op that a DVE row chain needs ever queues behind a
                    # copy that waits on a later apply.
                    rs_b = brdp.tile([128, 512], BF16, tag="brd", name="rs_b")
                    nc.scalar.activation(out=rs_b, in_=ps_rs[:, :],
                                         func=AF.Identity, scale=1.0)
                    mu_b = brdp.tile([128, 512], BF16, tag="brd", name="mu_b")
                    nc.scalar.activation(out=mu_b, in_=ps_mu[:, :],
                                         func=AF.Identity, scale=1.0)
                    brd.append((rs_b, mu_b))
                return brd

            def apply(job, brd, eng=None):
                src_fn, T, out_fn, sdt = job
                eng = eng or nc.vector
                # t1 rings are phase-disjoint with their pools' other users;
                # crucially NOT t32, whose sq tiles would gate later jobs'
                # stats on this apply's consumption.
                pool = rcbp if eng is nc.vector else tbf
                tag = "rcb" if eng is nc.vector else "tbf"
                for c in range(KC):
                    for th in range(T // 512):
                        sl = slice(th * 512, (th + 1) * 512)
                        rs_b, mu_b = brd[th]
                        src = src_fn(c)[:, sl]
                        src_f = src if sdt == BF16 else src.bitcast(F32)
                        t1 = pool.tile([128, 512], BF16, tag=tag, name="t1")
                        eng.tensor_tensor(out=t1, in0=src_f, in1=rs_b,
                                          op=ALU.mult)
                        eng.tensor_tensor(out=out_fn(c)[:, sl], in0=t1,
                                          in1=mu_b, op=ALU.subtract)

            # per-job stats+rows in FIFO order (pst/pln slots free in
            # emission order). The first job's apply goes out immediately
            # after the last rows so PE's consumer matmuls can start; the
            # remaining jobs' applies are returned as closures for the caller
            # to emit later (so they don't clog the DVE FIFO ahead of
            # latency-critical phase-B DVE ops).
            # Job0's apply runs on DVE right away (its consumers are
            # imminent); job1's apply runs on DVE after the last rows (DVE is
            # idle there while PE chews the first linears); job2's runs on
            # the Pool engine in the background (consumers ~150us later).
            brd_q = []
            for j, job in enumerate(jobs):
                brd_q.append(rows_bcast(job, stats(job)))
                if j == 0:
                    apply(job, brd_q[0])
            for j in range(1, len(jobs)):
                apply(jobs[j], brd_q[j],
                      eng=None if j == 1 else nc.gpsimd)

        def linear(out_fn, rhs_fn, wt, nk, fts, T, bias_col, act, kcw0=0,
                   ftw0=0, dve_bias=False):
            """out_fn(i)[128,T] = act(sum_kc wt[:,kc,:]^T rhs_fn(kc) + bias).

            fts: list of ft indices (output 128-chunks); wt cols indexed
            relative to ftw0."""
            nt = T // 512
            for i, ft in enumerate(fts):
                for t in range(nt):
                    sl = slice(t * 512, (t + 1) * 512)
                    ps = pln.tile([128, 512], F32, tag="lin", name="linps")
                    for kc in range(nk):
                        wslice = wt[:, kcw0 + kc,
                                    (ft - ftw0) * 128:(ft - ftw0 + 1) * 128]
                        nc.tensor.matmul(ps[:, :], wslice, rhs_fn(kc)[:, sl],
                                         start=(kc == 0), stop=(kc == nk - 1))
                    if dve_bias:
                        nc.vector.tensor_scalar(
                            out=out_fn(i)[:, sl], in0=ps[:, :],
                            scalar1=bc[:, bias_col + ft:bias_col + ft + 1],
                            scalar2=None, op0=ALU.add)
                    else:
                        nc.scalar.activation(
                            out=out_fn(i)[:, sl], in_=ps[:, :], func=act,
                            bias=bc[:, bias_col + ft:bias_col + ft + 1],
                            scale=1.0)

        def make_v(v_tile, src_fn, wv, bias_b, masked):
            for tt in range(NKT):
                for h0 in (0, 6):
                    ps = pln.tile([128, 512], F32, tag="lin", name="vps")
                    for kc in range(KC):
                        nc.tensor.matmul(ps[:, 0:384],
                                         src_fn(kc)[:, tt * 128:(tt + 1) * 128],
                                         wv[:, kc, h0 * 64:h0 * 64 + 384],
                                         start=(kc == 0), stop=(kc == KC - 1))
                    vrow = v_tile[:, tt, :].rearrange("p (h c) -> p h c", c=65)
                    nc.vector.tensor_tensor(
                        out=vrow[:, h0:h0 + 6, 0:64],
                        in0=ps[:, 0:384].rearrange("p (h c) -> p h c", c=64),
                        in1=bias_b[:, h0 * 64:h0 * 64 + 384].rearrange(
                            "p (h c) -> p h c", c=64),
                        op=ALU.add)
                    if masked:
                        nc.vector.tensor_scalar(
                            out=v_tile[:, tt, h0 * 65:(h0 + 6) * 65],
                            in0=v_tile[:, tt, h0 * 65:(h0 + 6) * 65],
                            scalar1=mm_sb[:, tt:tt + 1], scalar2=None,
                            op0=ALU.mult)

        def init_ones_cols(v_tile):
            for tt in range(NKT):
                vrow = v_tile[:, tt, :].rearrange("p (h c) -> p h c", c=65)
                nc.vector.tensor_copy(
                    out=vrow[:, :, 64:65],
                    in_=onesf[:, :].rearrange("p (h o) -> p h o", o=1))

        def norm_write(av, dst_ap):
            """Normalize AV psum rows 0:64 by the ones-row denominator in row
            64: copy the raw denom row to SBUF, broadcast it across 64
            partitions via a K=1 matmul, reciprocal into bf16, multiply."""
            den = rcpp.tile([1, 512], F32R, tag="rcp", name="den")
            nc.vector.tensor_copy(out=den, in_=av[64:65, :])
            ps_rc = pst.tile([64, 512], F32, tag="pst", name="ps_rc")
            nc.tensor.matmul(ps_rc[:, :], onerow[:, 0:64], den,
                             start=True, stop=True)
            rcb = rcbp.tile([64, 512], BF16, tag="rcb", name="rcb")
            with nc.allow_low_precision(reason="bf16 softmax denom"):
                nc.vector.reciprocal(out=rcb, in_=ps_rc[:, :])
            nc.vector.tensor_tensor(out=dst_ap, in0=av[0:64, :],
                                    in1=rcb, op=ALU.mult)

        def attention(kch_fn, v_tile, q_tile, dst, slot0, scale,
                      mem_dst_slot0=None, bg=None):
            """q_tile [128, KC, Q] bf16; writes dst[off:off+64, slot0+c, :].
            bg: optional list of closures; one is emitted after each head so
            their (DVE) work fills slack without clogging the FIFO."""
            for c in range(KC):
                kch = kch_fn(c)
                if bg:
                    bg.pop(0)()
                for hh in range(2):
                    h, off = 2 * c + hh, 64 * hh
                    av = pav.tile([65, 512], F32, tag="av", name="av")
                    for kt in range(NKT):
                        sc = psc.tile([128, 512], F32, tag="sc", name="sc")
                        nc.tensor.matmul(sc[:, :],
                                         kch[off:off + 64,
                                             kt * 128:(kt + 1) * 128],
                                         q_tile[off:off + 64, c, :],
                                         start=True, stop=True)
                        pt = ptp.tile([128, 512], BF16, tag="pt", name="pt")
                        nc.scalar.activation(out=pt, in_=sc[:, :], func=AF.Exp,
                                             scale=scale)
                        nc.tensor.matmul(av[0:65, :],
                                         v_tile[:, kt, h * 65:(h + 1) * 65],
                                         pt[:, :],
                                         start=(kt == 0), stop=(kt == NKT - 1))
                    if mem_dst_slot0 is not None:
                        scm = psc.tile([128, 512], F32, tag="sc", name="scm")
                        nc.tensor.matmul(scm[0:M, :], mk_sb[off:off + 64, c, :],
                                         q_tile[off:off + 64, c, :],
                                         start=True, stop=True)
                        pmt = ptp.tile([128, 512], BF16, tag="pt", name="pmt")
                        nc.scalar.activation(out=pmt[0:M, :], in_=scm[0:M, :],
                                             func=AF.Exp, scale=1.0)
                        av1 = pav.tile([65, 512], F32, tag="av", name="av1")
                        nc.tensor.matmul(av1[0:65, :],
                                         mv_sb[:, h * 65:(h + 1) * 65],
                                         pmt[0:M, :], start=True, stop=True)
                        norm_write(av, dst[off:off + 64, slot0 + c, :])
                        norm_write(av1,
                                   dst[off:off + 64, mem_dst_slot0 + c, :])
                    else:
                        norm_write(av, dst[off:off + 64, slot0 + c, :])

        def _finish():
            nc.sync.dma_start(
                out=outT[:, :].rearrange("(c p) t -> p c t", p=128),
                in_=xq32)

        # ======== phase A: x layernorm (in-place) ========
        if stop_after == "io":
            _finish()
            return
        # prefetch phase-B weight units before the LN group; the big f32
        # loads (V biases, residual copy of x) follow them on the sync queue
        wu_vs = wunit("vs", 0, KC, 0, E, "wu_vs")
        wu_qq = wunit("qk", 0, KC, 0, E, "wu_qq")
        wu_qk = wunit("qk", 0, KC, E, E, "wu_qk")
        bvb_s = consts.tile([128, E], F32)
        nc.sync.dma_start(out=bvb_s, in_=_row_bcast_dram(bv_self[:], 128))
        bvb_e = consts.tile([128, E], F32)
        nc.sync.dma_start(out=bvb_e, in_=_row_bcast_dram(bv_enc[:], 128))
        xq32 = asp.tile([128, KC, Q], F32, tag="as", name="xq32")
        nc.sync.dma_start(out=xq32,
                          in_=xqf[:, :].rearrange("(c p) t -> p c t", p=128))

        # x + both enc layernorms as one group: job j+1's stats matmuls keep
        # PE busy while DVE runs job j's row chains; all applies come last.
        xhat = x_sb
        stats_apply(
            (lambda c: x_sb[:, c, :], S, lambda c: xhat[:, c, :], BF16),
            (lambda c: enc_sb[0][:, c, :], SE,
             lambda c: enc_sb[0][:, c, :], BF16),
            (lambda c: enc_sb[1][:, c, :], SE,
             lambda c: enc_sb[1][:, c, :], BF16),
        )
        if stop_after == "A":
            _finish()
            return

        # ======== phase B: V, q, self+memory attention ========
        v_sb = vsbp.tile([128, NKT, H * 65], BF16, tag="vsb")
        init_ones_cols(v_sb)
        make_v(v_sb, lambda kc: xhat[:, kc, :], wu_vs, bvb_s, masked=False)
        qT = qp.tile([128, KC, Q], BF16, tag="q", name="qT")
        linear(lambda i: qT[:, i, :], lambda kc: xhat[:, kc, 0:Q], wu_qq,
               KC, list(range(KC)), Q, BC_Q, AF.Identity)
        preload(AF.Exp)
        gAB = g24.tile([128, 12, Q], BF16, tag="g24", name="gAB")

        def self_kchunk(c):
            kt = kchp.tile([128, S], BF16, tag="kch", name="kchs")
            linear(lambda i: kt, lambda kc: xhat[:, kc, :], wu_qk, KC, [c], S,
                   BC_K, AF.Identity, dve_bias=True)
            return kt

        attention(self_kchunk, v_sb, qT, gAB, 0, 1.0, mem_dst_slot0=KC)
        preload(AF.Sigmoid)
        if stop_after == "B":
            _finish()
            return

        # ======== phase B4: memory gate + attn_proj + residual ========
        wu_ma0 = wunit("ma", 0, KC, 0, E, "wu_ma0")
        wu_ma1 = wunit("ma", KC, KC, 0, E, "wu_ma1")
        aN = n12.tile([128, KC, Q], BF16, tag="n12", name="aN")
        for ft in range(KC):
            ps = pln.tile([128, 512], F32, tag="lin", name="maps")
            for kc in range(12):
                wt = wu_ma0 if kc < KC else wu_ma1
                nc.tensor.matmul(ps[:, :],
                                 wt[:, kc % KC, ft * 128:(ft + 1) * 128],
                                 gAB[:, kc, :], start=(kc == 0),
                                 stop=(kc == 11))
            al = tbf.tile([128, 512], BF16, tag="tbf", name="al")
            nc.scalar.activation(out=al, in_=ps[:, :], func=AF.Sigmoid,
                                 bias=bc[:, BC_MA + ft:BC_MA + ft + 1],
                                 scale=1.0)
            d = tbf.tile([128, 512], BF16, tag="tbf", name="d")
            nc.vector.tensor_tensor(out=d, in0=gAB[:, ft, :],
                                    in1=gAB[:, KC + ft, :], op=ALU.subtract)
            nc.vector.tensor_tensor(out=d, in0=al, in1=d, op=ALU.mult)
            nc.vector.tensor_tensor(out=aN[:, ft, :], in0=gAB[:, KC + ft, :],
                                    in1=d, op=ALU.add)
        wu_pr = wunit("proj", 0, KC, 0, E, "wu_pr")
        aT = aTp.tile([128, KC, Q], F32R, tag="aT", name="aT")
        ee = g24.tile([128, 18, Q], BF16, tag="g24", name="ee")
        for ft in range(KC):
            ps = pln.tile([128, 512], F32, tag="lin", name="prps")
            for kc in range(KC):
                nc.tensor.matmul(ps[:, :],
                                 wu_pr[:, kc, ft * 128:(ft + 1) * 128],
                                 aN[:, kc, :], start=(kc == 0),
                                 stop=(kc == KC - 1))
            nc.vector.scalar_tensor_tensor(
                out=aT[:, ft, :], in0=ps[:, :],
                scalar=bc[:, BC_PROJ + ft:BC_PROJ + ft + 1],
                in1=xq32[:, ft, :], op0=ALU.add, op1=ALU.add)
            nc.scalar.activation(out=ee[:, ft, :],
                                 in_=aT[:, ft, :].bitcast(F32),
                                 func=AF.Identity, scale=1.0)
        if stop_after == "B4":
            _finish()
            return

        # ======== phase C: LN(aT) -> qe ========
        preload(AF.Sqrt)
        # enc0's V projection depends only on the (already-normalized)
        # encoder: its 16 matmul chains keep PE busy through the serial aT
        # row chain. v_sb's ring1 WAR on self-attention readers is long past.
        wu_v0 = wunit("v", 0, KC, 0, E, "wu_v0")
        make_v(v_sb, lambda kc: enc_sb[0][:, kc, :], wu_v0, bvb_e,
               masked=True)
        hah = n12.tile([128, KC, Q], BF16, tag="n12", name="hah")
        stats_apply((lambda c: aT[:, c, :], Q, lambda c: hah[:, c, :], F32R))
        wu_q = wunit("q", 0, KC, 0, E, "wu_q")
        qeT = qp.tile([128, KC, Q], BF16, tag="q", name="qeT")
        linear(lambda i: qeT[:, i, :], lambda kc: hah[:, kc, :], wu_q,
               KC, list(range(KC)), Q, BC_FCQ, AF.Identity)
        preload(AF.Exp)

        asum = asp.tile([128, KC, Q], F32R, tag="as", name="asum")
        if stop_after == "C":
            _finish()
            return

        # ======== phase D: two cross-attentions ========
        for e in range(2):
            enc = enc_sb[e]
            if e == 1:
                wu_v = wunit("v", 0, KC, 0, E, "wu_v1")
                make_v(v_sb, lambda kc: enc[:, kc, :], wu_v, bvb_e,
                       masked=True)
            wu_k = wunit("k", 0, KC, 0, E, f"wu_k{e}")

            def enc_kchunk(c, _enc=enc, _wu_k=wu_k):
                kt = kchp.tile([128, SE], BF16, tag="kch", name="kche")
                linear(lambda i: kt, lambda kc: _enc[:, kc, :], _wu_k, KC,
                       [c], SE, BC_FCK, AF.Identity, dve_bias=True)
                return kt

            attention(enc_kchunk, v_sb, qeT, ee, KC, 0.125)

            wu_ep = wunit("ep", 0, KC, 0, E, f"wu_ep{e}")
            ep32 = n12.tile([128, KC, Q], F32, tag="n12", name="ep32")
            for ft in range(KC):
                ps = pln.tile([128, 512], F32, tag="lin", name="epps")
                for kc in range(KC):
                    nc.tensor.matmul(ps[:, :],
                                     wu_ep[:, kc, ft * 128:(ft + 1) * 128],
                                     ee[:, KC + kc, :], start=(kc == 0),
                                     stop=(kc == KC - 1))
                nc.vector.tensor_scalar(
                    out=ep32[:, ft, :], in0=ps[:, :],
                    scalar1=bc[:, BC_EP + ft:BC_EP + ft + 1], scalar2=None,
                    op0=ALU.add)
                nc.scalar.activation(out=ee[:, 2 * KC + ft, :],
                                     in_=ep32[:, ft, :], func=AF.Identity,
                                     scale=1.0)

            akey = "a1" if e == 0 else "a2"
            wu_g0 = wunit(akey, 0, KC, 0, E, f"wu_g0{e}")
            wu_g1 = wunit(akey, KC, KC, 0, E, f"wu_g1{e}")
            bcol0 = BC_A1 if e == 0 else BC_A2
            for ft in range(KC):
                ps = pln.tile([128, 512], F32, tag="lin", name="gps")
                for kc in range(12):
                    wt = wu_g0 if kc < KC else wu_g1
                    src = ee[:, kc, :] if kc < KC else ee[:, KC + kc, :]
                    nc.tensor.matmul(ps[:, :],
                                     wt[:, kc % KC, ft * 128:(ft + 1) * 128],
                                     src, start=(kc == 0), stop=(kc == 11))
                al = t32.tile([128, 512], F32, tag="t32", name="alE")
                nc.scalar.activation(out=al, in_=ps[:, :], func=AF.Sigmoid,
                                     bias=bc[:, bcol0 + ft:bcol0 + ft + 1],
                                     scale=1.0)
                # eg = ep + al*(aT - ep), all f32
                t = t32.tile([128, 512], F32, tag="t32", name="tE")
                nc.vector.tensor_tensor(out=t, in0=aT[:, ft, :].bitcast(F32),
                                        in1=ep32[:, ft, :], op=ALU.subtract)
                nc.vector.tensor_tensor(out=t, in0=al, in1=t, op=ALU.mult)
                if e == 0:
                    nc.gpsimd.tensor_tensor(out=asum[:, ft, :],
                                            in0=ep32[:, ft, :], in1=t,
                                            op=ALU.add)
                else:
                    eg = t32.tile([128, 512], F32, tag="t32", name="egE")
                    nc.vector.tensor_tensor(out=eg, in0=ep32[:, ft, :],
                                            in1=t, op=ALU.add)
                    nc.vector.tensor_tensor(out=asum[:, ft, :],
                                            in0=asum[:, ft, :].bitcast(F32),
                                            in1=eg, op=ALU.add)
            if stop_after == "D0" and e == 0:
                _finish()
                return
        if stop_after == "D":
            _finish()
            return

        # ======== phase E: MLP + final residual ========
        preload(AF.Sqrt)
        hm2 = n12.tile([128, KC, Q], BF16, tag="n12", name="hm2")
        stats_apply((lambda c: asum[:, c, :], Q, lambda c: hm2[:, c, :], F32R))
        preload(AF.Gelu_apprx_tanh)
        mstage = msp.tile([128, KC, Q], F32, tag="ms", name="mstage")
        for mh in range(2):
            mT = g24.tile([128, 12, Q], BF16, tag="g24", name="mT")
            for u in range(2):
                wu_fc = wunit("fc", 0, KC, (2 * mh + u) * E, E,
                              f"wu_fc{mh}{u}")
                linear(lambda i, _u=u: mT[:, 6 * _u + i, :],
                       lambda kc: hm2[:, kc, :], wu_fc, KC, list(range(KC)),
                       Q, BC_FC + 12 * mh + 6 * u, AF.Gelu_apprx_tanh,
                       ftw0=0)
            wu_p0 = wunit("pj", 12 * mh, KC, 0, E, f"wu_p0{mh}")
            wu_p1 = wunit("pj", 12 * mh + KC, KC, 0, E, f"wu_p1{mh}")
            for ft in range(KC):
                ps = pln.tile([128, 512], F32, tag="lin", name="pjps")
                for kc in range(12):
                    wt = wu_p0 if kc < KC else wu_p1
                    nc.tensor.matmul(ps[:, :],
                                     wt[:, kc % KC, ft * 128:(ft + 1) * 128],
                                     mT[:, kc, :], start=(kc == 0),
                                     stop=(kc == 11))
                if mh == 0:
                    nc.scalar.activation(out=mstage[:, ft, :], in_=ps[:, :],
                                         func=AF.Identity,
                                         bias=bc[:, BC_PJ + ft:BC_PJ + ft + 1],
                                         scale=1.0)
                else:
                    t = t32.tile([128, 512], F32, tag="t32", name="mo")
                    nc.vector.scalar_tensor_tensor(
                        out=t, in0=asum[:, ft, :].bitcast(F32),
                        scalar=float(1.0 / np.sqrt(2.0)), in1=ps[:, :],
                        op0=ALU.mult, op1=ALU.add)
                    ot = t32.tile([128, 512], F32, tag="t32", name="ot")
                    nc.vector.tensor_tensor(out=ot, in0=t, in1=mstage[:, ft, :],
                                            op=ALU.add)
                    nc.sync.dma_start(out=outT[ft * 128:(ft + 1) * 128, :],
                                      in_=ot)


_NC_CACHE = None


def _get_nc():
    global _NC_CACHE
    if _NC_CACHE is None:
        _NC_CACHE = build_program()
    return _NC_CACHE


def _pack_bias_cols(seg_biases):
    bcols = np.zeros((128, NB), np.float32)
    for col0, b in seg_biases:
        nf = b.shape[0] // 128
        bcols[:, col0:col0 + nf] = b.reshape(nf, 128).T
    return bcols


def _bf16(x):
    import ml_dtypes
    return np.asarray(x, np.float32).astype(ml_dtypes.bfloat16).view(np.uint16)


def prepare_in_maps(x, encoder_features, mask_encoder, ln1_g, ln1_b, ln2_g,
                    ln2_b, c_attn_w, c_attn_b, attn_proj_w, attn_proj_b,
                    memory_features, mem_attn_w, mem_attn_b, mem_alpha_w,
                    mem_alpha_b, fcq_w, fcq_b, fck_w, fck_b, fcv_w, fcv_b,
                    enc_proj_w, enc_proj_b, fc_alpha1_w, fc_alpha1_b,
                    fc_alpha2_w, fc_alpha2_b, mlp_fc_w, mlp_fc_b, mlp_proj_w,
                    mlp_proj_b):
    f32 = np.float32
    x = np.asarray(x, f32)
    encoder_features = np.asarray(encoder_features, f32)

    # ---- fold LN gains/biases into consumer weights ----
    g1 = np.asarray(ln1_g, f32); b1 = np.asarray(ln1_b, f32)
    g2 = np.asarray(ln2_g, f32); b2 = np.asarray(ln2_b, f32)

    def fold(w, b, g, lb):
        w = np.asarray(w, f32); b = np.asarray(b, f32)
        return (w * g[:, None]).astype(f32), (lb @ w + b).astype(f32)

    w_qkv, b_qkv = fold(c_attn_w, c_attn_b, g1, b1)
    w_fcq, b_fcq = fold(fcq_w, fcq_b, g1, b1)
    w_fck, b_fck = fold(fck_w, fck_b, g1, b1)
    w_fcv, b_fcv = fold(fcv_w, fcv_b, g1, b1)
    w_mfc, b_mfc = fold(mlp_fc_w, mlp_fc_b, g2, b2)

    # ---- memory slots (batch independent) ----
    mem = (np.asarray(memory_features, f32)[0] @ np.asarray(mem_attn_w, f32)
           + np.asarray(mem_attn_b, f32))          # [M, 2E]
    mk = mem[:, :E].reshape(M, H, D)
    mv = mem[:, E:].reshape(M, H, D)
    mkT = np.zeros((128, KC, M), f32)
    mvA = np.zeros((M, H * 65), f32)
    for h in range(H):
        c, off = divmod(h, 2)
        mkT[off * 64:(off + 1) * 64, c, :] = mk[:, h, :].T
        mvA[:, h * 65:h * 65 + 64] = mv[:, h, :]
        mvA[:, h * 65 + 64] = 1.0

    bcols = _pack_bias_cols([
        (BC_Q, b_qkv[0:E]), (BC_K, b_qkv[E:2 * E]),
        (BC_PROJ, np.asarray(attn_proj_b, f32)),
        (BC_MA, np.asarray(mem_alpha_b, f32)),
        (BC_FCQ, b_fcq), (BC_FCK, b_fck),
        (BC_EP, np.asarray(enc_proj_b, f32)),
        (BC_A1, np.asarray(fc_alpha1_b, f32)),
        (BC_A2, np.asarray(fc_alpha2_b, f32)),
        (BC_FC, b_mfc), (BC_PJ, np.asarray(mlp_proj_b, f32)),
    ])

    keep = (~np.asarray(mask_encoder, bool)[:, 0, 0, :]).astype(f32)  # [B, SE]

    common = dict(
        mkT=_bf16(mkT), mvA=_bf16(mvA),
        w_qk=_bf16(w_qkv[:, 0:2 * E]),
        w_vs=_bf16(w_qkv[:, 2 * E:3 * E]),
        w_proj=_bf16(attn_proj_w),
        w_ma=_bf16(mem_alpha_w),
        w_q=_bf16(w_fcq), w_k=_bf16(w_fck), w_v=_bf16(w_fcv),
        w_ep=_bf16(enc_proj_w),
        w_a1=_bf16(fc_alpha1_w),
        w_a2=_bf16(fc_alpha2_w),
        w_fc=_bf16(w_mfc), w_pj=_bf16(mlp_proj_w),
        bcols=bcols,
        bv_self=np.ascontiguousarray(b_qkv[2 * E:3 * E]),
        bv_enc=b_fcv,
    )

    in_maps = []
    for core in range(8):
        b, half = divmod(core, 2)
        xTb = np.ascontiguousarray(x[b].T)                       # [E, S]
        xrot = np.concatenate([xTb[:, half * Q:], xTb[:, :half * Q]], axis=1)
        m = dict(common)
        m["xT"] = _bf16(xrot)
        m["xqf"] = np.ascontiguousarray(xrot[:, 0:Q])
        m["encT"] = _bf16(encoder_features[b].transpose(0, 2, 1))
        m["maskmul"] = np.ascontiguousarray(keep[b].reshape(NKT, 128).T)
        in_maps.append(m)
    return in_maps


def kernel(**inputs):
    in_maps = prepare_in_maps(**inputs)
    nc = _get_nc()
    res = run_bass_kernel_spmd(nc, in_maps, core_ids=list(range(8)))

    global _LAST_IN_MAPS
    _LAST_IN_MAPS = in_maps

    y = np.empty((B, S, E), np.float32)
    for core in range(8):
        b, half = divmod(core, 2)
        y[b, half * Q:(half + 1) * Q, :] = res.results[core]["outT"].T
    return y


_LAST_IN_MAPS = None


def _make_runner(nc, in_maps):
    """Build a jitted 8-core runner; returns run(n) -> wall seconds for n
    pipelined calls with device-resident inputs and donated outputs."""
    import time
    import jax
    from jax.sharding import Mesh, PartitionSpec
    from jax.experimental.shard_map import shard_map
    import concourse.mybir as mybir_
    from concourse import bass2jax

    n_cores = 8
    bass2jax.install_neuronx_cc_hook()

    in_names, out_names, out_avals, zero_outs = [], [], [], []
    partition_name = nc.partition_id_tensor.name if nc.partition_id_tensor else None
    for alloc in nc.m.functions[0].allocations:
        if not isinstance(alloc, mybir_.MemoryLocationSet):
            continue
        name = alloc.memorylocations[0].name
        if alloc.kind == "ExternalInput":
            if name != partition_name:
                in_names.append(name)
        elif alloc.kind == "ExternalOutput":
            out_avals.append(jax.core.ShapedArray(
                tuple(alloc.tensor_shape), mybir_.dt.np(alloc.dtype)))
            zero_outs.append(np.zeros(tuple(alloc.tensor_shape),
                                      mybir_.dt.np(alloc.dtype)))
            out_names.append(name)
    n_params = len(in_names)
    n_outs = len(out_avals)
    all_in_names = in_names + out_names + ([partition_name] if partition_name else [])
    donate = tuple(range(n_params, n_params + n_outs))

    def _body(*args):
        operands = list(args)
        if partition_name is not None:
            operands.append(bass2jax.partition_id_tensor())
        return tuple(bass2jax._bass_exec_p.bind(
            *operands, out_avals=tuple(out_avals), in_names=tuple(all_in_names),
            out_names=tuple(out_names), lowering_input_output_aliases=(),
            sim_require_finite=True, sim_require_nnan=True, nc=nc))

    devices = jax.devices()[:n_cores]
    mesh = Mesh(np.asarray(devices), ("core",))
    fn = jax.jit(shard_map(_body, mesh=mesh,
                           in_specs=(PartitionSpec("core"),) * (n_params + n_outs),
                           out_specs=(PartitionSpec("core"),) * n_outs,
                           check_rep=False),
                 donate_argnums=donate, keep_unused=True)
    sh = jax.sharding.NamedSharding(mesh, PartitionSpec("core"))
    concat_in = [jax.device_put(
        np.concatenate([np.asarray(in_maps[c][nm]) for c in range(n_cores)], 0), sh)
        for nm in in_names]

    def zeros():
        return [jax.device_put(
            np.zeros((n_cores * z.shape[0], *z.shape[1:]), z.dtype), sh)
            for z in zero_outs]

    def run(n):
        o = tuple(zeros())
        o = fn(*concat_in, *o)
        jax.block_until_ready(o)
        t0 = time.perf_counter()
        for _ in range(n):
            o = fn(*concat_in, *o)
        jax.block_until_ready(o)
        return time.perf_counter() - t0

    return run


def profile_exec_ns(n_hot=12, n_cold=2, nc=None, in_maps=None, quiet=False,
                    samples=5):
    """Estimate per-invocation device time by timing pipelined repeats of the
    jitted 8-core executable; median of `samples` cold/hot estimates."""
    if in_maps is None:
        in_maps = _LAST_IN_MAPS
    if in_maps is None:
        return None
    if nc is None:
        nc = _get_nc()
    run = _make_runner(nc, in_maps)
    ests = []
    for _ in range(samples):
        tc = run(n_cold)
        th = run(n_hot)
        ests.append((th - tc) / (n_hot - n_cold))
    ests.sort()
    per = ests[len(ests) // 2]
    if not quiet:
        print(f"pipelined per-call estimates (us): "
              + " ".join(f"{e*1e6:.0f}" for e in ests))
    return int(per * 1e9)


def slope_exec_ns(n=16, iters=4, in_maps=None, quiet=False):
    """Per-execution device time via the body-repeat slope: build the kernel
    with the body emitted once and 4x in one NEFF; the wall-time difference
    per call over 3 extra bodies cancels all host/dispatch overhead.
    Interleaves A/B measurements to reject machine-state drift."""
    if in_maps is None:
        in_maps = _LAST_IN_MAPS
    if in_maps is None:
        return None
    run1 = _make_runner(_get_nc(), in_maps)
    run4 = _make_runner(build_program(repeat=4), in_maps)
    d1, d4 = [], []
    for _ in range(iters):
        d1.append(min(run1(n), run1(n)))
        d4.append(min(run4(n), run4(n)))
    d1.sort(), d4.sort()
    m1, m4 = d1[len(d1) // 2], d4[len(d4) // 2]
    per = (m4 - m1) / 3 / n
    if not quiet:
        print("slope walls (ms): K1 "
              + " ".join(f"{t*1e3:.1f}" for t in d1) + "  K4 "
              + " ".join(f"{t*1e3:.1f}" for t in d4))
    return int(per * 1e9)

